# revision 38
# baseline (speedup 1.0000x reference)
"""Trainium2 Bass kernel for multi-head attention (B=4, N=2048, C=768, H=12).

Sharding: zero-collective data parallel across 8 NeuronCores. Core c handles
batch b=c//2 and query rows [(c%2)*1024, +1024). K/V are computed over all
2048 keys of the batch (softmax over keys is permutation invariant, so each
core receives its x with its own query rows rolled to the front). Pairs of
cores duplicate only the cheap K/V projection; there are no collectives.

P2 is a three-engine near-tie (PE ~21us/head, Act exp ~20us, DVE reduce_max
~23us), so the design keeps all three balanced and saturated:

  P0: load x [2048,768] in 8+8 chunks on the SP/Act DMA queues, PE-transpose
      to xT [768,2048]; PSUM->SBUF copies batched 768-wide, split Act/DVE.
  P1: QKV projections, w_qkv streamed in six 384-col chunks (bufs=2).
      Phase order q -> k -> v so the k-staging writes (to DRAM, for P2's
      rotating ka buffers) never sit ahead of later weight loads in a DMA
      queue. Head 0's entire S-max pass runs HERE (K=64 matmuls straight off
      the k-staging SBUF tile, paced between k/v matmuls) so its ~23us of
      reduce work hides under P1 instead of serializing at P2 start.
      All zero/const lanes (q-tile row 64 + pads, ka pads, vhat ones col)
      are gpsimd memsets - no DRAM zero traffic.
  P2: softmax-shift m per row need NOT be the exact row max: any value in
      [max, max+10] keeps exp/sum/recip in range (softmax is shift
      invariant; bf16 has f32 exponent range). So some 1024-key chunks
      compute ln(sum(exp(S))) on the Act engine (fused exp+accum_out, then
      Ln) instead of reduce_max on DVE - LSE >= max and <= max+ln(1024) -
      rebalancing work between the two bottleneck engines (n_lse per pass:
      normal heads 2, tail-0's double passes 5/6).
      Tail: S~^T matmul pair -> ONE 1024-wide exp(8x) -> ONE 1024-wide bf16
      AV matmul into av[65,1024] (row 64 = denominators via vhat ones col).
      Normalization: Act copies av PSUM->SBUF, DVE reciprocal_approx_fast,
      gpsimd partition_broadcast + multiply into outNT.
  P3: projection out = outNT.T @ w_proj, PSUM->SBUF copies split Act/DVE.

Matmul compute dtype float32r ~ tf32; rel err ~2.8e-3 vs the 2e-2 gate.
"""

import numpy as np

NQ = 1024  # queries per core
NK = 2048  # keys per core
CD = 768
H = 12
D = 64
P = 128

_CACHE = {}


def _build():
    from contextlib import ExitStack

    import concourse.bacc as bacc
    import concourse.mybir as mybir
    import concourse.tile as tile

    f32 = mybir.dt.float32
    f32r = mybir.dt.float32r
    bf16 = mybir.dt.bfloat16
    EXP = mybir.ActivationFunctionType.Exp
    AX = mybir.AxisListType.X
    MAX = mybir.AluOpType.max
    SHR = mybir.AluOpType.logical_shift_right
    SUB = mybir.AluOpType.subtract
    MUL = mybir.AluOpType.mult
    u32 = mybir.dt.uint32

    nc = bacc.Bacc("TRN2", target_bir_lowering=False, debug=False, num_devices=8)

    x_ap = nc.dram_tensor("x", [NK, CD], f32, kind="ExternalInput").ap()
    wqkv_ap = nc.dram_tensor("w_qkv", [CD, 3 * CD], f32, kind="ExternalInput").ap()
    wproj_ap = nc.dram_tensor("w_proj", [CD, CD], f32, kind="ExternalInput").ap()
    ident_ap = nc.dram_tensor("ident", [P, P], f32, kind="ExternalInput").ap()

    kstg_ap = nc.dram_tensor("kstg", [H, D, NK], f32r).ap()  # heads 1..11 used
    mstg_ap = nc.dram_tensor("mstg", [H, NQ], f32r).ap()
    out_ap = nc.dram_tensor("out", [NQ, CD], f32, kind="ExternalOutput").ap()
    import os
    DBG = bool(os.environ.get("KDBG"))
    if DBG:
        dmx_ap = nc.dram_tensor("dmx", [2, P, 16], f32, kind="ExternalOutput").ap()
        dq_ap = nc.dram_tensor("dq", [2, P, NQ], f32r, kind="ExternalOutput").ap()
        dk_ap = nc.dram_tensor("dk", [P, NK], f32r, kind="ExternalOutput").ap()
        dv_ap = nc.dram_tensor("dv", [P, 16, H, D + 1], bf16, kind="ExternalOutput").ap()
        do_ap = nc.dram_tensor("do", [P, 6, NQ], f32r, kind="ExternalOutput").ap()
        dm_ap = nc.dram_tensor("dm", [H, NQ], f32r, kind="ExternalOutput").ap()

    x_t = x_ap.rearrange("(t p) c -> p t c", p=P)  # [128, 16, 768]
    wqkv_t = wqkv_ap.rearrange("(a p) n -> p a n", p=P)  # [128, 6, 2304]
    wproj_t = wproj_ap.rearrange("(a p) n -> p a n", p=P)  # [128, 6, 768]
    out_t = out_ap.rearrange("(t p) c -> p t c", p=P)  # [128, 8, 768]

    # how many of each head's 16 S-max chunks go to Act (LSE) vs DVE (max).
    # head 0's pass runs in P1 where the LSE scratch pool doesn't exist yet.
    N_LSE = {0: 0, 1: 4, 2: 5}
    N_LSE_DEFAULT = 1

    with tile.TileContext(nc) as tc, ExitStack() as ctx:
        # ---- persistent pools (order fixes 1KB alignment of MM operands) ----
        pers = ctx.enter_context(tc.tile_pool(name="pers", bufs=1))
        ident_sb = pers.tile([P, P], f32, tag="ident", padded_shape=[P, 1024])
        nc.sync.dma_start(ident_sb[:], ident_ap)

        qpool = ctx.enter_context(tc.tile_pool(name="qt", bufs=1))
        q_tiles = [
            qpool.tile([P, NQ], f32r, tag=f"q{h}", name=f"q{h}") for h in range(H)
        ]
        # q rows 64..127 (shift lane + pads) must be 0 before pass-1 S matmuls
        for h in range(H):
            nc.gpsimd.memset(q_tiles[h][D:P, :].bitcast(f32), 0.0)

        wpp = ctx.enter_context(tc.tile_pool(name="wp", bufs=1))
        wp_r = wpp.tile([P, 6, CD], bf16, tag="wp")

        vpool = ctx.enter_context(tc.tile_pool(name="vhat", bufs=1))
        # padded so the per-partition byte count is a multiple of 1KB and the
        # pools that follow stay 1KB-aligned (misaligned moving operands run
        # ~57% slower on the PE)
        vhat = vpool.tile(
            [P, 16, H, D + 1], bf16, tag="vhat", padded_shape=[P, 16, H, D + 8]
        )
        nc.gpsimd.memset(vhat[:, :, :, D], 1.0)  # denominators lane

        # head 0's k-tilde lives in SBUF through P1 and tail-0 of P2: rows
        # 0:63 = k dims, row 64 = -1 (shift lane), rows 65+ = 0
        kst0p = ctx.enter_context(tc.tile_pool(name="kst0", bufs=1))
        kst0 = kst0p.tile([P, NK], f32r, tag="kst0")
        # engine APs need 32-aligned partition bases: zero 64:128, then -1 row
        nc.gpsimd.memset(kst0[D:P, :].bitcast(f32), 0.0)
        nc.gpsimd.memset(kst0[D : D + 1, :].bitcast(f32), -1.0)

        # mx pool persists into P2 (head-0 maxes are produced in P1); every
        # tile padded to 1KB so later pool starts stay aligned. Must be
        # created before the xT pool (pool release is stack-ordered).
        mxp = ctx.enter_context(tc.tile_pool(name="mx", bufs=2))

        from contextlib import ExitStack as _ES

        ctx_xT = _ES()
        xT_pool = ctx_xT.enter_context(tc.tile_pool(name="xT", bufs=1))
        xT = xT_pool.tile([P, 6, NK], f32r, tag="xT")  # [C-chunk part, kc, row]

        # ================= P0: load x, transpose =================
        with (
            tc.tile_pool(name="xn", bufs=1) as xnp,
            tc.tile_pool(name="ps0", bufs=3, space="PSUM") as ps0,
        ):
            xn = xnp.tile([P, 16, CD], f32, tag="xn")
            for tq in range(16):
                eng = nc.sync if tq % 2 == 0 else nc.scalar
                eng.dma_start(xn[:, tq : tq + 1, :], x_t[:, tq : tq + 1, :])
            # ~4us of throwaway matmuls while the x DMAs stream: PE-transposes
            # don't count as HAM activity, so without these the first ~3.4us
            # of P1 matmuls run at the cold 1.2GHz clock
            warm = ps0.tile([P, P], f32, tag="warm", bufs=1)
            for _w in range(32):
                nc.tensor.matmul(
                    warm[:], ident_sb[:], ident_sb[:], start=True, stop=True
                )
            # pre-load the Exp ACT table set here (~2.7us) so the first real
            # exp in P2 doesn't pay it on the critical path
            wtbl = xnp.tile([P, 1], bf16, tag="wtbl")
            nc.scalar.activation(wtbl[:], ident_sb[:, 0:1], EXP, scale=1.0)
            for t in range(16):
                pst = ps0.tile([P, CD], f32, tag="tr")
                for kc in range(6):
                    nc.tensor.transpose(
                        pst[:, kc * P : (kc + 1) * P],
                        xn[:, t, kc * P : (kc + 1) * P],
                        ident_sb[:],
                    )
                # one 768-wide PSUM->SBUF copy per x row-chunk (f32->f32r
                # rounds); dst strided across the 6 kc planes of xT
                dst = xT[:, :, t * P : (t + 1) * P]
                if t % 2 == 0:
                    nc.scalar.copy(dst, pst[:].rearrange("p (a b) -> p a b", a=6))
                else:
                    nc.vector.tensor_copy(
                        dst, pst[:].rearrange("p (a b) -> p a b", a=6)
                    )

        # ================= P1: QKV projections =================
        def mxtile(shape, dtype, nm):
            return mxp.tile(shape, dtype, tag=nm[:3], name=nm, padded_shape=[P, 256])

        max2_tiles = {}
        lse_tiles = {}

        s_ps_pool = [None]  # set per phase: P1 uses psX1, P2 uses merged pool
        trash_pool = [None]  # set in P2 (LSE chunks only run there)
        ka_src = {0: kst0}  # h -> SBUF tile holding k-tilde rows (0:64 at least)

        def s_begin_core(h):
            max2_tiles[h] = mxtile([P, 16], f32, f"mx2_{h}")
            lse_tiles[h] = mxtile([P, 16], f32, f"lse_{h}")

        def n_lse_of(h):
            return N_LSE.get(h, N_LSE_DEFAULT)

        def s_step(h, i, k64=False):
            """Pass-1 chunk i (qt=i//2, keys half=i%2) of head h's shift calc.
            Chunks with i >= 16-n_lse go to Act as ln-sum-exp (an upper bound
            on the chunk max within +ln(1024), equally valid as a softmax
            shift); the rest are exact row-max chunks on DVE."""
            qa, ka = q_tiles[h], ka_src[h]
            qt, half2 = i // 2, i % 2
            ps_s = s_ps_pool[0].tile([P, NK // 2], f32, tag="ps", name=f"s{h}_{i}")
            kp = D if k64 else P
            for mc in range(2):
                m0 = half2 * 1024 + mc * 512
                nc.tensor.matmul(
                    ps_s[:, mc * 512 : (mc + 1) * 512],
                    qa[0:kp, qt * P : (qt + 1) * P],
                    ka[0:kp, m0 : m0 + 512],
                    start=True,
                    stop=True,
                )
            if i >= 16 - n_lse_of(h):
                trash = trash_pool[0].tile(
                    [P, NK // 2], bf16, tag="tr", name=f"tr{h}_{i}"
                )
                nc.scalar.activation(
                    trash[:], ps_s[:], EXP, scale=1.0,
                    accum_out=lse_tiles[h][:, i : i + 1],
                )
            else:
                nc.vector.reduce_max(
                    max2_tiles[h][:, i : i + 1], ps_s[:], axis=AX
                )

        def s_end(h):
            """Combine the 16 chunk results into per-row shifts, scatter into
            q row 64 via a transposing DRAM bounce."""
            qa, max2, lse = q_tiles[h], max2_tiles[h], lse_tiles[h]
            nl = n_lse_of(h)
            if nl:
                # m = floor(log2(sum))*ln2 (in [ln(sum)-0.7, ln(sum)]) via the
                # f32 exponent bits - avoids an Ln activation, whose table set
                # differs from Exp's and forces ~2.7us table swaps on Act
                lse_b = lse[:, 16 - nl : 16].bitcast(u32)
                nc.vector.tensor_scalar(
                    out=lse_b, in0=lse_b, scalar1=23, scalar2=None, op0=SHR
                )
                nc.vector.tensor_scalar(
                    out=max2[:, 16 - nl : 16],
                    in0=lse_b,
                    scalar1=127.0,
                    scalar2=0.6931472,
                    op0=SUB,
                    op1=MUL,
                )
            maxr = mxtile([P, 8], f32r, f"mxr_{h}")
            nc.vector.tensor_tensor(
                out=maxr[:], in0=max2[:, 0:16:2], in1=max2[:, 1:16:2], op=MAX
            )
            # scatter shifts into q-tilde row 64 (elem (p,qt) -> col qt*128+p).
            # Both DMAs ride the gpsimd software queue: their data-waits would
            # head-of-line-block the sync/scalar queues that carry ka loads,
            # and the gpsimd engine only runs the cheap broadcasts in P2.
            nc.gpsimd.dma_start(mstg_ap[h].rearrange("(a b) -> b a", b=P), maxr[:])
            nc.gpsimd.dma_start(qa[D : D + 1, :], mstg_ap[h])

        with (
            tc.tile_pool(name="wq", bufs=2) as wqp,
            tc.tile_pool(name="wtmp", bufs=2) as wtp,
            tc.tile_pool(name="kst", bufs=2) as kstp,
            tc.tile_pool(name="ps1", bufs=2, space="PSUM") as ps1,
            tc.tile_pool(name="ps1v", bufs=2, space="PSUM") as ps1v,
            tc.tile_pool(name="psX1", bufs=1, space="PSUM") as psX1,
        ):
            s_ps_pool[0] = psX1

            def load_chunk(c):
                wq_r = wqp.tile([P, 6, 384], f32r, tag="wq")
                for a in range(6):
                    wtmp = wtp.tile([P, 384], f32, tag="wt")
                    eng = nc.sync if a % 2 == 0 else nc.scalar
                    eng.dma_start(wtmp[:], wqkv_t[:, a, 384 * c : 384 * (c + 1)])
                    nc.vector.tensor_copy(wq_r[:, a, :], wtmp[:])
                return wq_r

            wq_r0 = load_chunk(0)
            # w_proj load queued after the first chunk so it never gates P1
            for a in range(6):
                wtmp3t = wtp.tile([P, CD], f32, tag="wt3", bufs=1)
                eng = nc.sync if a % 2 == 0 else nc.scalar
                eng.dma_start(wtmp3t[:], wproj_t[:, a, :])
                nc.vector.tensor_copy(wp_r[:, a, :], wtmp3t[:])

            # ---- q phase (chunks 0,1) ----
            for c in range(2):
                wq_r = wq_r0 if c == 0 else load_chunk(c)
                for jl in range(3):
                    j = 3 * c + jl  # heads 2j, 2j+1
                    ps = ps1.tile([P, NQ], f32, tag="p1", name=f"qb{j}")
                    for kc in range(6):
                        for ncn in range(2):
                            nc.tensor.matmul(
                                ps[:, ncn * 512 : (ncn + 1) * 512],
                                wq_r[:, kc, jl * P : (jl + 1) * P],
                                xT[:, kc, ncn * 512 : (ncn + 1) * 512],
                                start=(kc == 0),
                                stop=(kc == 5),
                            )
                    nc.scalar.copy(q_tiles[2 * j][0:D, :], ps[0:D, :])
                    nc.vector.tensor_copy(q_tiles[2 * j + 1][0:D, :], ps[D : 2 * D, :])

            # ---- k phase (chunks 2,3), head-0 S-pass paced between blocks ----
            s_begin_core(0)
            s0_next = [0]

            def s0_pump(n):
                for _ in range(n):
                    if s0_next[0] < 16:
                        s_step(0, s0_next[0], k64=True)
                        s0_next[0] += 1

            for c in range(2, 4):
                wq_r = load_chunk(c)
                for jl in range(3):
                    j = 3 * (c - 2) + jl  # heads 2j, 2j+1
                    pss = [
                        ps1.tile([P, NQ], f32, tag="p1", name=f"kb{j}_{n}")
                        for n in range(2)
                    ]
                    for kc in range(6):
                        for ncn in range(4):
                            nc.tensor.matmul(
                                pss[ncn // 2][:, (ncn % 2) * 512 : (ncn % 2 + 1) * 512],
                                wq_r[:, kc, jl * P : (jl + 1) * P],
                                xT[:, kc, ncn * 512 : (ncn + 1) * 512],
                                start=(kc == 0),
                                stop=(kc == 5),
                            )
                    for i in range(2):  # head h = 2j+i
                        h = 2 * j + i
                        if h == 0:
                            dst = kst0
                        else:
                            dst = kstp.tile(
                                [D, NK], f32r, tag="kst", name=f"kst{h}"
                            )
                        r0 = i * D
                        nc.scalar.copy(dst[0:D, 0:NQ], pss[0][r0 : r0 + D, :])
                        nc.vector.tensor_copy(dst[0:D, NQ:NK], pss[1][r0 : r0 + D, :])
                        if h != 0:
                            eng = nc.sync if h % 2 == 0 else nc.scalar
                            eng.dma_start(kstg_ap[h], dst[0:D, :])
                    if j > 0:
                        s0_pump(2)

            # ---- v phase (chunks 4,5) ----
            for c in range(4, 6):
                wq_r = load_chunk(c)
                vc = c - 4
                for t in range(16):
                    ps = ps1v.tile([P, 384], f32, tag="vp")
                    for kc in range(6):
                        nc.tensor.matmul(
                            ps[:],
                            xT[:, kc, t * P : (t + 1) * P],
                            wq_r[:, kc, 0:384],
                            start=(kc == 0),
                            stop=(kc == 5),
                        )
                    ceng = t % 2
                    dst = vhat[:, t, 6 * vc : 6 * (vc + 1), 0:D]
                    src = ps[:].rearrange("p (h d) -> p h d", d=D)
                    if ceng == 0:
                        nc.scalar.copy(dst, src)
                    else:
                        nc.vector.tensor_copy(dst, src)
                    if c == 4 and t % 2 == 1:
                        s0_pump(1)
            s0_pump(16)  # any leftovers (10 + 8 pump slots >= 16 steps)
            s_end(0)
            if DBG:
                nc.gpsimd.dma_start(dq_ap[0], q_tiles[0][:])
                nc.gpsimd.dma_start(dq_ap[1], q_tiles[3][:])
                nc.gpsimd.dma_start(dk_ap, kst0[:])
                for _t in range(16):
                    nc.gpsimd.dma_start(
                        dv_ap[:, _t], vhat[:, _t, :, 0 : D + 1]
                    )

        ctx_xT.close()  # xT dead after P1 - free 48KB/partition for P2

        # ================= P2: attention per head =================
        opool = ctx.enter_context(tc.tile_pool(name="outNT", bufs=1))
        outNT = opool.tile([P, 6, NQ], bf16, tag="outNT")
        with (
            tc.tile_pool(name="ka", bufs=4) as kap,
            tc.tile_pool(name="at", bufs=3) as atp,
            tc.tile_pool(name="scr", bufs=2) as scrp,
            tc.tile_pool(name="trash", bufs=2) as trashp,
            tc.tile_pool(name="nrm", bufs=2) as nrmp,
            tc.tile_pool(name="psT", bufs=3, space="PSUM") as psT,
            tc.tile_pool(name="psAV", bufs=1, space="PSUM") as psAV,
        ):
            s_ps_pool[0] = psT
            trash_pool[0] = trashp

            # zero/init the four rotating ka buffers once; later loads only
            # touch rows 0:63, so row 64 (-1) and the pad rows persist
            ka_ring = []
            for b in range(4):
                kaz = kap.tile([P, NK], f32r, tag="ka", name=f"ka{b}")
                nc.gpsimd.memset(kaz[D:P, :].bitcast(f32), 0.0)
                nc.gpsimd.memset(kaz[D : D + 1, :].bitcast(f32), -1.0)
                ka_ring.append(kaz)

            def ka_load(h):
                ka = ka_ring[(h - 1) % 4]
                eng = nc.sync if h % 2 == 0 else nc.scalar
                eng.dma_start(ka[0:D, :], kstg_ap[h])
                ka_src[h] = ka

            av_tiles = {}

            def t_begin(h):
                av_tiles[h] = psAV.tile(
                    [D + 1, NQ], f32, tag="av", name=f"av{h}"
                )

            at_tiles = {}

            def t_st(h, mt):
                # S~^T matmul pair into one 2-bank tile + ONE 1024-wide exp;
                # the AV matmul of mt-1 issues after the ST pair of mt so the
                # in-order PE queue never waits on the Act engine
                qa, ka = q_tiles[h], ka_src[h]
                ps_st = psT.tile([P, NQ], f32, tag="ps", name=f"st{h}_{mt}")
                for ncn in range(2):
                    nc.tensor.matmul(
                        ps_st[:, ncn * 512 : (ncn + 1) * 512],
                        ka[:, mt * P : (mt + 1) * P],
                        qa[:, ncn * 512 : (ncn + 1) * 512],
                        start=True,
                        stop=True,
                    )
                at = atp.tile([P, NQ], bf16, tag="at")
                nc.scalar.activation(at[:], ps_st[:], EXP, scale=8.0)
                at_tiles[(h, mt)] = at

            def t_av(h, mt):
                # matmul output must fit one PSUM bank -> two 512-wide halves
                at = at_tiles.pop((h, mt))
                for ncn in range(2):
                    nc.tensor.matmul(
                        av_tiles[h][:, ncn * 512 : (ncn + 1) * 512],
                        vhat[:, mt, h, 0 : D + 1],
                        at[:, ncn * 512 : (ncn + 1) * 512],
                        start=(mt == 0),
                        stop=(mt == 15),
                    )

            pending_mul = {}

            def t_end(h):
                ps_av = av_tiles[h]
                scr = scrp.tile([D + 1, NQ], f32, tag="scr")
                nc.scalar.copy(scr[:], ps_av[:])
                # one [64,NQ] tile serves den -> recip (in-place, safe: the
                # 8-stage DVE pipe writes elem i after reading i) -> broadcast
                # (self-overlapping row 0 rewrite is idempotent).
                # reciprocal_approx_fast mishandles base-partition-64 input,
                # so the Act copy lands the denominator row at partition 0.
                rb = nrmp.tile([D, NQ], f32, tag="rb")
                nc.scalar.copy(rb[0:1, :], ps_av[D : D + 1, :])
                nc.vector.reciprocal_approx_fast(rb[0:1, :], rb[0:1, :])
                nc.gpsimd.partition_broadcast(rb[:], rb[0:1, :])
                pending_mul[h] = (scr, rb)

            def t_end_mul(h):
                # the outNT multiply, deferred into the NEXT tail so it never
                # sits in the DVE FIFO ahead of that tail's s-pass reduces
                scr, rb = pending_mul.pop(h)
                r0 = D * (h % 2)
                nc.vector.tensor_mul(
                    outNT[r0 : r0 + D, h // 2, :], scr[0:D, :], rb[:]
                )

            # software pipeline, two heads deep: during tail h the S-pass of
            # head h+2 runs one chunk per iteration, so its shift-bounce DMA
            # has the whole of tail h+1 to land before tail h+2 needs it.
            # tail(0) carries the S-passes of BOTH heads 1 and 2.
            # s-pass chunk counts per mt iteration: compressed into the
            # first 12 iterations so s_end (and its bounce DMA, whose
            # data-wait head-of-line-blocks the gpsimd queue) fires several
            # iterations before the tail ends
            SPLAN = [2, 2, 2, 2, 1, 1, 1, 1, 1, 1, 1, 1, 0, 0, 0, 0]
            for h in range(H):
                t_begin(h)
                if h == 0:
                    ka_load(1)
                    ka_load(2)
                    ka_load(3)
                    s_begin_core(1)
                    s_begin_core(2)
                else:
                    if h + 3 < H:
                        ka_load(h + 3)
                    if h + 2 < H:
                        s_begin_core(h + 2)
                snext = 0
                s2next = [0]
                for mt in range(16):
                    t_st(h, mt)
                    if h == 0:
                        if mt < 8:
                            s_step(1, 2 * mt)
                            s_step(1, 2 * mt + 1)
                            if mt == 7:
                                s_end(1)
                        elif mt < 14:
                            for _ in range(3 if mt < 12 else 2):
                                if s2next[0] < 16:
                                    s_step(2, s2next[0])
                                    s2next[0] += 1
                            if mt == 13:
                                s_end(2)
                    elif h + 2 < H:
                        for _ in range(SPLAN[mt]):
                            if snext < 16:
                                s_step(h + 2, snext)
                                snext += 1
                        if mt == 11:
                            s_end(h + 2)
                    if mt == 2 and (h - 1) in pending_mul:
                        t_end_mul(h - 1)
                    if mt >= 1:
                        t_av(h, mt - 1)
                t_av(h, 15)
                t_end(h)
            t_end_mul(H - 1)
            if DBG:
                nc.gpsimd.dma_start(do_ap, outNT[:])
                nc.gpsimd.dma_start(dm_ap, mstg_ap)

        # ================= P3: output projection =================
        with (
            tc.tile_pool(name="ob", bufs=3) as obp,
            tc.tile_pool(name="psP", bufs=3, space="PSUM") as psP,
        ):
            for nt in range(8):
                ps = psP.tile([P, CD], f32, tag="pj")
                for kc in range(6):
                    for c0, cn in ((0, 512), (512, 256)):
                        nc.tensor.matmul(
                            ps[:, c0 : c0 + cn],
                            outNT[:, kc, nt * P : (nt + 1) * P],
                            wp_r[:, kc, c0 : c0 + cn],
                            start=(kc == 0),
                            stop=(kc == 5),
                        )
                osb = obp.tile([P, CD], f32, tag="ob")
                if nt % 2 == 0:
                    nc.scalar.copy(osb[:], ps[:])
                else:
                    nc.vector.tensor_copy(osb[:], ps[:])
                oeng = nc.sync if nt % 2 == 0 else nc.scalar
                oeng.dma_start(out_t[:, nt, :], osb[:])

    nc.compile()
    return nc


def _in_maps(x, w_qkv, w_proj):
    ident = np.eye(P, dtype=np.float32)
    maps = []
    for c in range(8):
        b, qh = c // 2, c % 2
        xb = np.roll(x[b], -qh * NQ, axis=0) if qh else x[b]
        maps.append(
            {
                "x": np.ascontiguousarray(xb, dtype=np.float32),
                "w_qkv": np.ascontiguousarray(w_qkv, dtype=np.float32),
                "w_proj": np.ascontiguousarray(w_proj, dtype=np.float32),
                "ident": ident,
            }
        )
    return maps


LAST = {}


def kernel(x, w_qkv, w_proj):
    import os

    from concourse import bass_utils

    if "nc" not in _CACHE:
        _CACHE["nc"] = _build()
    nc = _CACHE["nc"]
    kwargs = {}
    if os.environ.get("KERNEL_TRACE"):
        kwargs["trace"] = True
        if os.environ.get("KERNEL_TRACE_DIR"):
            kwargs["tmpdir"] = os.environ["KERNEL_TRACE_DIR"]
    res = bass_utils.run_bass_kernel_spmd(
        nc, _in_maps(x, w_qkv, w_proj), core_ids=list(range(8)), **kwargs
    )
    LAST["exec_time_ns"] = res.exec_time_ns
    out = np.empty((4, 2048, CD), np.float32)
    for c in range(8):
        b, qh = c // 2, c % 2
        out[b, qh * NQ : (qh + 1) * NQ] = res.results[c]["out"]
    return out
